# revision 5
# baseline (speedup 1.0000x reference)
"""Axial (row/column) attention block — nn_DBCAFM_26242250179366.

Device implementation: 8-way SPMD on the 8 trn2 NeuronCores via jax pmap
(axon PJRT).  Sharding per the hint: data-parallel over B (4 batch
elements x 2 cores each); within a batch-element pair, the output rows
H are split in half.  Each shard computes the full row(W)-direction
attention (needed because out1/out2 couple all rows through v_w/qk_w)
plus the half column(H)-direction attention for its output rows — no
cross-device communication is required.

Shapes (hardcoded): B=4, C=64, H=128, W=128, nh=4, hd=16.
"""

import numpy as np

B, C, H, W, NH = 4, 64, 128, 128, 4
HD = C // NH
HH = H // 2  # rows per shard


def _build():
    import jax
    import jax.numpy as jnp

    def _segsum(x):
        # x: (..., L) -> (..., L, L) bidirectional segment sums
        cs = jnp.cumsum(x, axis=-1)
        d = cs[..., :, None] - cs[..., None, :]
        L = x.shape[-1]
        i = jnp.arange(L)[:, None]
        j = jnp.arange(L)[None, :]
        return jnp.where(i >= j, d, -d)

    def _layernorm(x, g, b, eps=1e-5):
        m = x.mean(-1, keepdims=True)
        v = ((x - m) ** 2).mean(-1, keepdims=True)
        return (x - m) * jax.lax.rsqrt(v + eps) * g + b

    def _gelu_tanh(x):
        # tanh-approx gelu (exact erf variant needs a second ACT table set)
        return 0.5 * x * (1.0 + jnp.tanh(0.7978845608028654 * (x + 0.044715 * x * x * x)))

    def _rot(x):
        x1 = x[..., ::2]
        x2 = x[..., 1::2]
        return jnp.stack([-x2, x1], axis=-1).reshape(x.shape)

    def shard_fn(x, y, th, h0, da, db, dw1_w, dw1_b, dw2_w, dw2_b, qw, qb,
                 kw, kb, vw, vb, lepe_w, lepe_b, ow, ob,
                 n1_g, n1_b, ffn_w1, ffn_b1, ffn_w2, ffn_b2, n2_g, n2_b,
                 cos, sin):
        # x,y,th: (C,H,W) for this shard's batch element; h0: row offset (0/64)
        scaling = HD ** (-0.5)

        # ---- dynamic gated fusion (1x1 convs) ----
        fusion = jnp.concatenate([y, th], axis=0)                   # (2C,H,W)
        hid = jax.nn.relu(jnp.einsum('chw,oc->ohw', fusion, dw1_w)
                          + dw1_b[:, None, None])
        logits = jnp.einsum('chw,oc->ohw', hid, dw2_w) + dw2_b[:, None, None]
        wts = jax.nn.softmax(logits, axis=0)                        # (2,H,W)
        fused = y * wts[0:1] + th * wts[1:2]

        xh = jnp.transpose(x, (1, 2, 0))                            # (H,W,C)
        fkv = jnp.transpose(fused, (1, 2, 0))

        q = xh @ qw + qb
        k = (fkv @ kw + kb) * scaling
        v = fkv @ vw + vb                                           # (H,W,C)

        # depthwise 5x5 conv position term on v (NHWC), batch dim of 1
        lepe = jax.lax.conv_general_dilated(
            v[None], lepe_w, (1, 1), ((2, 2), (2, 2)),
            dimension_numbers=('NHWC', 'HWIO', 'NHWC'),
            feature_group_count=C)[0] + lepe_b                      # (H,W,C)

        q5 = q.reshape(H, W, NH, HD).transpose(2, 0, 1, 3)          # (nh,H,W,hd)
        k5 = k.reshape(H, W, NH, HD).transpose(2, 0, 1, 3)
        v5 = v.reshape(H, W, NH, HD).transpose(2, 0, 1, 3)

        qr = q5 * cos + _rot(q5) * sin
        kr = k5 * cos + _rot(k5) * sin

        # ---- data-dependent decay masks (da/db precomputed host-side) ----
        mask_w = _segsum(da.reshape(H, W, NH).transpose(0, 2, 1))   # (H,nh,W,W)
        mask_h = _segsum(db.reshape(H, W, NH).transpose(1, 2, 0))   # (W,nh,H,H)


        # ---- full row (width-direction) attention ----
        s_w = jnp.einsum('nhid,nhjd->hnij', qr, kr) + mask_w        # (H,nh,W,W)
        qk_w = jax.nn.softmax(s_w, axis=-1)
        v_w = jnp.einsum('hnij,nhjd->hnid', qk_w, v5)               # (H,nh,W,hd)

        # ---- half column (height-direction) attention: queries h0:h0+HH ----
        qr_h = jax.lax.dynamic_slice_in_dim(qr, h0, HH, axis=1)     # (nh,HH,W,hd)
        mask_h_q = jax.lax.dynamic_slice_in_dim(mask_h, h0, HH, axis=2)
        s_h = jnp.einsum('nhwd,ngwd->wnhg', qr_h, kr) + mask_h_q    # (W,nh,HH,H)
        qk_h = jax.nn.softmax(s_h, axis=-1)

        # out1 rows h0:h0+HH = qk_h @ v_w
        out1 = jnp.einsum('wnhg,gnwd->wnhd', qk_h, v_w)             # (W,nh,HH,hd)
        out1 = out1.transpose(2, 0, 1, 3).reshape(HH, W, C)

        # v_h for query rows h0:h0+HH, all columns
        v_h = jnp.einsum('wnhg,ngwd->wnhd', qk_h, v5)               # (W,nh,HH,hd)

        # out2 rows h0:h0+HH = qk_w[h0:h0+HH] @ v_h
        qk_w_half = jax.lax.dynamic_slice_in_dim(qk_w, h0, HH, axis=0)
        out2 = jnp.einsum('hnij,jnhd->hnid', qk_w_half, v_h)        # (HH,nh,W,hd)
        out2 = out2.transpose(0, 2, 1, 3).reshape(HH, W, C)

        lepe_half = jax.lax.dynamic_slice_in_dim(lepe, h0, HH, axis=0)
        xh_half = jax.lax.dynamic_slice_in_dim(xh, h0, HH, axis=0)

        out = 0.5 * out1 + 0.5 * out2 + lepe_half                   # (HH,W,C)
        out = out @ ow + ob
        out = _layernorm(xh_half + out, n1_g, n1_b)
        ffn = _gelu_tanh(out @ ffn_w1 + ffn_b1) @ ffn_w2 + ffn_b2
        out = _layernorm(out + ffn, n2_g, n2_b)
        return jnp.transpose(out, (2, 0, 1))                        # (C,HH,W)

    n_weights = 22  # dw1_w .. n2_b (dt/A handled host-side)
    in_axes = (0, 0, 0, 0, 0, 0) + (None,) * n_weights + (None, None)
    pfn = jax.pmap(shard_fn, in_axes=in_axes, devices=jax.devices()[:8])
    return jax, jnp, pfn


_CACHE = {}


def _get_pfn():
    if 'pfn' not in _CACHE:
        _CACHE['pfn'] = _build()
    return _CACHE['pfn']


def kernel(x, y, th, dw1_w, dw1_b, dw2_w, dw2_b, qw, qb, kw, kb, vw, vb,
           lepe_w, lepe_b, dt_w, dt_bias, A_log, ow, ob,
           n1_g, n1_b, ffn_w1, ffn_b1, ffn_w2, ffn_b2, n2_g, n2_b):
    jax, jnp, pfn = _get_pfn()
    f32 = np.float32

    # 2D RoPE tables (host-precomputed constants)
    angle = np.repeat(1.0 / 10000 ** np.linspace(0.0, 1.0, HD // 2), 2)
    idx = np.arange(H * W, dtype=np.float64)
    ph = idx[:, None] * angle[None, :]
    cos = np.cos(ph).reshape(H, W, HD).astype(f32)
    sin = np.sin(ph).reshape(H, W, HD).astype(f32)

    # decay rates da/db (host: tiny matmul + softplus; avoids device ln)
    xf = np.asarray(x, f32)
    xt = np.ascontiguousarray(xf.reshape(B, C, H * W).transpose(0, 2, 1))
    dt = xt.reshape(B, H * W, NH, HD) @ np.asarray(dt_w, f32)       # (B,S,nh,2)
    A = -np.exp(np.asarray(A_log, f32))
    sp = lambda t: np.logaddexp(t, 0.0).astype(f32)
    da = (sp(dt[..., 0] + dt_bias) * A).astype(f32)                 # (B,S,nh)
    db = (sp(dt[..., 1] + dt_bias) * A).astype(f32)

    # shard inputs: shard s -> batch s//2, row-half s%2
    xs = xf[np.arange(8) // 2]                                      # (8,C,H,W)
    ys = np.asarray(y, f32)[np.arange(8) // 2]
    ths = np.asarray(th, f32)[np.arange(8) // 2]
    h0s = (np.arange(8) % 2 * HH).astype(np.int32)

    das = da[np.arange(8) // 2]
    dbs = db[np.arange(8) // 2]

    weights = [np.asarray(w, f32) for w in
               (dw1_w, dw1_b, dw2_w, dw2_b, qw, qb, kw, kb, vw, vb,
                lepe_w, lepe_b, ow, ob,
                n1_g, n1_b, ffn_w1, ffn_b1, ffn_w2, ffn_b2, n2_g, n2_b)]

    out_shards = pfn(xs, ys, ths, h0s, das, dbs, *weights, cos, sin)  # (8,C,HH,W)
    out_shards = np.asarray(out_shards)

    out = np.empty((B, C, H, W), dtype=f32)
    for s in range(8):
        b, half = s // 2, s % 2
        out[b, :, half * HH:(half + 1) * HH, :] = out_shards[s]
    return out
